# revision 10
# baseline (speedup 1.0000x reference)
"""Trainium2 Bass kernel for the AttentiveModule problem.

Reference computation (per batch element b, S=1024, D=512):
    att   = aspect @ inp.T / sqrt(len)                # [S,S]
    exp   = att * mask[:, None]                       # row mask (query dim)
    att_n = exp / (exp.sum(-1, keepdims=True) + 1e-4) # linear normalize
    w     = att_n @ inp                               # [S,D]
    ffn_inp = w + (inp + aspect) * mask[:, None]
    o1    = relu(ffn_inp @ w1.T + b1)
    o2    = relu(o1 @ w2.T + b2)
    final = 2*ffn_inp + o2
    out   = final / ||final||_2(axis=-1)

Sharding: data-parallel over batch, one batch element per NeuronCore (8 cores).

Key algebraic restructuring (host prep is O(B*S*D), trivial vs the O(S^2*D)
device work):
  - The row mask and the linear normalization commute with the second matmul:
      w[s,:] = g[s] * (raw @ inp)[s,:],  raw = aspect @ inp.T
      g[s] = mask[s] / (mask[s]*rowsum_raw[s] + 1e-4*sqrt(len))
    rowsum_raw[s] = aspect[s,:] @ inp.sum(0) is a host-side f64 matvec, so the
    device needs no [S,S] elementwise work nor partition-dim reductions.
  - g (and the factor 2 of final = 2*ffn_inp + o2) is folded into the aspect
    operand of the first matmul, whose output attnT = (2*g*att_n).T lands
    directly in the stationary-operand layout the second matmul needs.
  - The FFN2 bias enters PSUM via a K=1 ones x b2 matmul; w1/2 compensates
    the folded factor 2.
  - Inputs are packed on the host into [128, X] images matching the SBUF
    destination layout exactly, split into chunks issued in consumption
    order on two HWDGE queues so phase A starts as early as possible.

Matmul operands are bf16 (fp32 PSUM accumulation); the residual path stays
fp32 end to end.
"""

import os
import sys

for _p in ("/opt/trn_rl_repo", "/opt/pypackages"):
    if os.path.isdir(_p) and _p not in sys.path:
        sys.path.append(_p)

import numpy as np
import ml_dtypes

BF16 = ml_dtypes.bfloat16

B, S, D = 8, 1024, 512
N_CORES = 8
P = 128                     # SBUF partitions
SB = S // P                 # 8 s-blocks of 128
DB = D // P                 # 4 d-blocks of 128
NF = 512                    # matmul moving free dim (one fp32 PSUM bank)
SH = S // NF                # 2 s-halves

# --- packed input layouts (element column offsets) -----------------------
# gA (bf16): aTg s-half 0 | inpT tb-chunks 0..3 | aTg s-half 1 | inpT 4..7
#   aTg half:   [P, DB*NF]   (db-major)
#   inpT chunk: [P, DB*P]    (db-major, all d-blocks for one t-block)
A_ATG0 = 0
A_INPT0 = DB * NF                       # 2048
A_ATG1 = A_INPT0 + 4 * DB * P           # 4096
A_INPT4 = A_ATG1 + DB * NF              # 6144
A_COLS = A_INPT4 + 4 * DB * P           # 8192
# gB1 (bf16): inpN (sb-major)
B1_COLS = SB * D            # 4096
# gB2 (bf16): w1th | w2t | b2 row | ones row | identb
B2_W1 = 0
B2_W2 = DB * D              # 2048
B2_B2 = B2_W2 + DB * D      # 4096  (row 0 only)
B2_ONES = B2_B2 + D         # 4608  (row 0 only)
B2_IDB = B2_ONES + P        # 4736
B2_COLS = B2_IDB + P        # 4864
# gF (f32): resm2 (sb-major) | b1cb
F_RES = 0
F_B1 = SB * D               # 4096
F_COLS = F_B1 + DB          # 4100

_COMPILED = None


def _build():
    import concourse.bacc as bacc
    import concourse.tile as tile
    import concourse.mybir as mybir

    f32 = mybir.dt.float32
    bf16 = mybir.dt.bfloat16
    AF = mybir.ActivationFunctionType
    ALU = mybir.AluOpType

    nc = bacc.Bacc("TRN2", target_bir_lowering=False, debug=False,
                   num_devices=N_CORES)

    packA = nc.dram_tensor("packA", [P, A_COLS], bf16, kind="ExternalInput").ap()
    packB1 = nc.dram_tensor("packB1", [P, B1_COLS], bf16, kind="ExternalInput").ap()
    packB2 = nc.dram_tensor("packB2", [P, B2_COLS], bf16, kind="ExternalInput").ap()
    packF = nc.dram_tensor("packF", [P, F_COLS], f32, kind="ExternalInput").ap()
    out = nc.dram_tensor("out", [S, D], f32, kind="ExternalOutput").ap()

    with tile.TileContext(nc) as tc:
        import contextlib
        ctx = contextlib.ExitStack()
        with ctx:
            big = ctx.enter_context(tc.tile_pool(name="big", bufs=1))
            psA = ctx.enter_context(tc.tile_pool(name="psA", bufs=4, space="PSUM"))
            psT = ctx.enter_context(tc.tile_pool(name="psT", bufs=4, space="PSUM"))
            work = ctx.enter_context(tc.tile_pool(name="work", bufs=3))

            # phase-A loads on the Sync HWDGE queue, in consumption order
            gA = big.tile([P, A_COLS], bf16, name="gA")
            CHUNKS = [(A_ATG0, A_INPT0),
                      (A_INPT0, A_INPT0 + DB * P),
                      (A_INPT0 + DB * P, A_INPT0 + 2 * DB * P),
                      (A_INPT0 + 2 * DB * P, A_INPT0 + 3 * DB * P),
                      (A_INPT0 + 3 * DB * P, A_ATG1),
                      (A_ATG1, A_INPT4),
                      (A_INPT4, A_COLS)]
            for lo, hi in CHUNKS:
                nc.sync.dma_start(gA[:, lo:hi], packA[:, lo:hi])
            # everything else on the Scalar HWDGE queue
            gB1 = big.tile([P, B1_COLS], bf16, name="gB1")
            nc.scalar.dma_start(gB1[:], packB1[:])
            gF = big.tile([P, F_COLS], f32, name="gF")
            nc.scalar.dma_start(gF[:], packF[:])
            gB2 = big.tile([P, B2_COLS], bf16, name="gB2")
            nc.scalar.dma_start(gB2[:], packB2[:])

            def inpT(tb, db):       # [P, P]: lhsT for t-block tb, d-block db
                base = A_INPT0 if tb < 4 else A_INPT4
                off = base + (tb % 4) * DB * P + db * P
                return gA[:, off: off + P]

            def aTg(h, db):         # [P, NF]: aTg[d-block db, s-half h]
                base = A_ATG0 if h == 0 else A_ATG1
                off = base + db * NF
                return gA[:, off: off + NF]

            def inpN(sb):           # [P, D]
                return gB1[:, sb * D: (sb + 1) * D]

            def w1th(db):           # [P, D]
                return gB2[:, B2_W1 + db * D: B2_W1 + (db + 1) * D]

            def w2t(eb):
                return gB2[:, B2_W2 + eb * D: B2_W2 + (eb + 1) * D]

            b2row = gB2[0:1, B2_B2: B2_B2 + D]
            onesrow = gB2[0:1, B2_ONES: B2_ONES + P]
            identb = gB2[:, B2_IDB: B2_IDB + P]

            def resm2(sb):          # [P, D] f32
                return gF[:, F_RES + sb * D: F_RES + (sb + 1) * D]

            def b1col(eb):          # [P, 1] f32
                return gF[:, F_B1 + eb: F_B1 + eb + 1]

            # ---- phase A: attnT[t,s] = sum_d inpT[d,t] * aTg[d,s] --------
            attnT_sb = []
            for tb in range(SB):
                at_t = big.tile([P, S], bf16, name=f"attnT_sb{tb}")
                attnT_sb.append(at_t)
            gi = 0
            for h in range(SH):
                for tb in range(SB):
                    ps = psA.tile([P, NF], f32, name="psA_t", tag="psA")
                    for db in range(DB):
                        nc.tensor.matmul(
                            ps[:],
                            inpT(tb, db),
                            aTg(h, db),
                            start=(db == 0),
                            stop=(db == DB - 1),
                        )
                    dst = attnT_sb[tb][:, h * NF:(h + 1) * NF]
                    if gi % 2 == 0:
                        nc.scalar.activation(dst, ps[:], AF.Copy)
                    else:
                        nc.vector.tensor_copy(dst, ps[:])
                    gi += 1

            # ---- phase B + C interleaved ---------------------------------
            # B: F2 = 2*ffn_inp = attnT.T @ inpN + resm2  (add on DVE)
            # C: ffnT2 = F2.T via PE transposes into one [P,512] PSUM tile
            #    per s-block, evacuated with a single strided copy
            F2_sb = []
            F2b_sb = []
            for sb in range(SB):
                f2 = big.tile([P, D], f32, name=f"F2_sb{sb}")
                F2_sb.append(f2)
                f2b = big.tile([P, D], bf16, name=f"F2b_sb{sb}")
                F2b_sb.append(f2b)
            ffnT2 = big.tile([P, DB * S], bf16, name="ffnT2")   # db-major [db*S + s]
            ffnT2_v = ffnT2.rearrange("p (db s) -> p db s", db=DB)
            for sb in range(SB):
                ps = psA.tile([P, NF], f32, name="psB_t", tag="psA")
                for tb in range(SB):
                    nc.tensor.matmul(
                        ps[:],
                        attnT_sb[tb][:, sb * P:(sb + 1) * P],
                        inpN(tb),
                        start=(tb == 0),
                        stop=(tb == SB - 1),
                    )
                nc.vector.tensor_add(F2_sb[sb][:], ps[:], resm2(sb))
                nc.vector.tensor_copy(F2b_sb[sb][:], F2_sb[sb][:])
                for db in range(DB):
                    pst = psT.tile([P, P], bf16, name="psT_t", tag="psT")
                    nc.tensor.transpose(
                        pst[:], F2b_sb[sb][:, db * P:(db + 1) * P], identb)
                    dst = ffnT2[:, db * S + sb * P: db * S + (sb + 1) * P]
                    if (sb + db) % 2 == 0:
                        nc.scalar.activation(dst, pst[:], AF.Copy)
                    else:
                        nc.vector.tensor_copy(dst, pst[:])

            # ---- phase D: o1T = relu(w1th.T @ ffnT2 + b1) [e, s] ---------
            o1T_sb = []
            for eb in range(DB):
                t = big.tile([P, S], bf16, name=f"o1T_sb{eb}")
                o1T_sb.append(t)
            for eb in range(DB):
                for h in range(SH):
                    ps = psA.tile([P, NF], f32, name="psD_t", tag="psA")
                    for db in range(DB):
                        nc.tensor.matmul(
                            ps[:],
                            w1th(db)[:, eb * P:(eb + 1) * P],
                            ffnT2[:, db * S + h * NF: db * S + (h + 1) * NF],
                            start=(db == 0),
                            stop=(db == DB - 1),
                        )
                    nc.scalar.activation(
                        o1T_sb[eb][:, h * NF:(h + 1) * NF], ps[:], AF.Relu,
                        bias=b1col(eb), scale=1.0)

            # ---- phase E: o2, final, normalize, store --------------------
            for sb in range(SB):
                ps = psA.tile([P, NF], f32, name="psE_t", tag="psA")
                nc.tensor.matmul(ps[:], onesrow, b2row, start=True, stop=False)
                for eb in range(DB):
                    nc.tensor.matmul(
                        ps[:],
                        o1T_sb[eb][:, sb * P:(sb + 1) * P],
                        w2t(eb),
                        start=False,
                        stop=(eb == DB - 1),
                    )
                o2 = work.tile([P, D], f32, name="o2_t", tag="o2")
                nc.scalar.activation(o2[:], ps[:], AF.Relu)
                fin = work.tile([P, D], f32, name="fin_t", tag="fin")
                nc.vector.tensor_add(fin[:], o2[:], F2_sb[sb][:])
                sq = work.tile([P, D], f32, name="sq_t", tag="sq")
                ss = work.tile([P, 1], f32, name="ss_t", tag="ss")
                nc.scalar.activation(sq[:], fin[:], AF.Square,
                                     accum_out=ss[:])
                rn = work.tile([P, 1], f32, name="rn_t", tag="rn")
                nc.scalar.activation(rn[:], ss[:], AF.Sqrt)
                rr = work.tile([P, 1], f32, name="rr_t", tag="rr")
                nc.vector.reciprocal(rr[:], rn[:])
                ot = work.tile([P, D], f32, name="ot_t", tag="ot")
                nc.vector.tensor_scalar_mul(ot[:], fin[:], rr[:])
                if sb % 2 == 0:
                    nc.sync.dma_start(out[sb * P:(sb + 1) * P, :], ot[:])
                else:
                    nc.scalar.dma_start(out[sb * P:(sb + 1) * P, :], ot[:])

    nc.compile()
    return nc


def _get_compiled():
    global _COMPILED
    if _COMPILED is None:
        _COMPILED = _build()
    return _COMPILED


def _host_prep(inp, inp_len, aspect, w1, b1, w2, b2):
    inp = np.asarray(inp, dtype=np.float32)
    aspect = np.asarray(aspect, dtype=np.float32)
    inp_len = np.asarray(inp_len, dtype=np.float32)
    w1 = np.asarray(w1, dtype=np.float32)
    b1 = np.asarray(b1, dtype=np.float32)
    w2 = np.asarray(w2, dtype=np.float32)
    b2 = np.asarray(b2, dtype=np.float32)

    packB2 = np.zeros((P, B2_COLS), dtype=BF16)
    w1th = (w1.T * 0.5).astype(BF16)                 # [d, e]
    w2t = w2.T.astype(BF16)                          # [e, f]
    for db in range(DB):
        packB2[:, B2_W1 + db * D: B2_W1 + (db + 1) * D] = \
            w1th[db * P:(db + 1) * P, :]
        packB2[:, B2_W2 + db * D: B2_W2 + (db + 1) * D] = \
            w2t[db * P:(db + 1) * P, :]
    packB2[0, B2_B2: B2_B2 + D] = b2.astype(BF16)
    packB2[0, B2_ONES: B2_ONES + P] = np.ones(P, dtype=BF16)
    packB2[:, B2_IDB: B2_IDB + P] = np.eye(P).astype(BF16)

    b1cb = b1.reshape(DB, P).T.astype(np.float32)    # [P, DB]

    in_maps = []
    for bidx in range(B):
        x = inp[bidx].astype(np.float64)             # [S, D]
        a = aspect[bidx].astype(np.float64)
        ln = float(inp_len[bidx])
        scale = np.sqrt(ln)
        mask = (np.arange(S) < int(ln)).astype(np.float64)
        rowsum = a @ x.sum(axis=0)
        g = mask / (mask * rowsum + 1e-4 * scale)
        aTg2 = ((a * (2.0 * g)[:, None]).T).astype(BF16)   # [D, S]
        xT = x.T.astype(BF16)                              # [D, S]
        resm2 = 2.0 * (x + a) * mask[:, None]              # [S, D]

        pA = np.empty((P, A_COLS), dtype=BF16)
        for h, base in ((0, A_ATG0), (1, A_ATG1)):
            for db in range(DB):
                pA[:, base + db * NF: base + (db + 1) * NF] = \
                    aTg2[db * P:(db + 1) * P, h * NF:(h + 1) * NF]
        for tb in range(SB):
            base = (A_INPT0 if tb < 4 else A_INPT4) + (tb % 4) * DB * P
            for db in range(DB):
                pA[:, base + db * P: base + (db + 1) * P] = \
                    xT[db * P:(db + 1) * P, tb * P:(tb + 1) * P]

        pB1 = np.empty((P, B1_COLS), dtype=BF16)
        xb = x.astype(BF16)
        for sb in range(SB):
            pB1[:, sb * D:(sb + 1) * D] = xb[sb * P:(sb + 1) * P, :]

        pF = np.zeros((P, F_COLS), dtype=np.float32)
        r32 = resm2.astype(np.float32)
        for sb in range(SB):
            pF[:, F_RES + sb * D: F_RES + (sb + 1) * D] = \
                r32[sb * P:(sb + 1) * P, :]
        pF[:, F_B1: F_B1 + DB] = b1cb

        in_maps.append({"packA": pA, "packB1": pB1, "packB2": packB2,
                        "packF": pF})
    return in_maps


def kernel(inp, inp_len, aspect, w1, b1, w2, b2):
    from concourse.bass_utils import run_bass_kernel_spmd

    nc = _get_compiled()
    in_maps = _host_prep(inp, inp_len, aspect, w1, b1, w2, b2)
    res = run_bass_kernel_spmd(nc, in_maps, core_ids=list(range(N_CORES)))
    return np.stack([res.results[i]["out"] for i in range(N_CORES)], axis=0)


# revision 12
# speedup vs baseline: 1.1198x; 1.1198x over previous
"""Trainium2 Bass kernel for the AttentiveModule problem.

Reference computation (per batch element b, S=1024, D=512):
    att   = aspect @ inp.T / sqrt(len)                # [S,S]
    exp   = att * mask[:, None]                       # row mask (query dim)
    att_n = exp / (exp.sum(-1, keepdims=True) + 1e-4) # linear normalize
    w     = att_n @ inp                               # [S,D]
    ffn_inp = w + (inp + aspect) * mask[:, None]
    o1    = relu(ffn_inp @ w1.T + b1)
    o2    = relu(o1 @ w2.T + b2)
    final = 2*ffn_inp + o2
    out   = final / ||final||_2(axis=-1)

Sharding: data-parallel over batch, one batch element per NeuronCore (8 cores).

Key algebraic restructuring (host prep is O(B*S*D), trivial vs the O(S^2*D)
device work):
  - The row mask and the linear normalization commute with the second matmul:
      w[s,:] = g[s] * (raw @ inp)[s,:],  raw = aspect @ inp.T
      g[s] = mask[s] / (mask[s]*rowsum_raw[s] + 1e-4*sqrt(len))
    rowsum_raw[s] = aspect[s,:] @ inp.sum(0) is a host-side f64 matvec, so the
    device needs no [S,S] elementwise work nor partition-dim reductions.
  - g (and the factor 2 of final = 2*ffn_inp + o2) is folded into the aspect
    operand of the first matmul, whose output attnT = (2*g*att_n).T lands
    directly in the stationary-operand layout the second matmul needs.
  - The FFN2 bias enters PSUM via a K=1 ones x b2 matmul; w1/2 compensates
    the folded factor 2.
  - Inputs are packed on the host into [128, X] images matching the SBUF
    destination layout exactly, split into chunks issued in consumption
    order on two HWDGE queues so phase A starts as early as possible.

Matmul operands are bf16 (fp32 PSUM accumulation); the residual path stays
fp32 end to end.
"""

import os
import sys

for _p in ("/opt/trn_rl_repo", "/opt/pypackages"):
    if os.path.isdir(_p) and _p not in sys.path:
        sys.path.append(_p)

import numpy as np
import ml_dtypes

BF16 = ml_dtypes.bfloat16

B, S, D = 8, 1024, 512
N_CORES = 8
P = 128                     # SBUF partitions
SB = S // P                 # 8 s-blocks of 128
DB = D // P                 # 4 d-blocks of 128
NF = 512                    # matmul moving free dim (one fp32 PSUM bank)
SH = S // NF                # 2 s-halves

# --- packed input layouts (element column offsets) -----------------------
# gA (bf16): aTg s-half 0 | inpT tb-chunks 0..3 | aTg s-half 1 | inpT 4..7
#   aTg half:   [P, DB*NF]   (db-major)
#   inpT chunk: [P, DB*P]    (db-major, all d-blocks for one t-block)
A_ATG0 = 0
A_INPT0 = DB * NF                       # 2048
A_ATG1 = A_INPT0 + 4 * DB * P           # 4096
A_INPT4 = A_ATG1 + DB * NF              # 6144
A_COLS = A_INPT4 + 4 * DB * P           # 8192
# gB1 (bf16): inpN (sb-major)
B1_COLS = SB * D            # 4096
# gB2 (bf16): w1th | w2t | b2 row | ones row | identb
B2_W1 = 0
B2_W2 = DB * D              # 2048
B2_B2 = B2_W2 + DB * D      # 4096  (row 0 only)
B2_ONES = B2_B2 + D         # 4608  (row 0 only)
B2_IDB = B2_ONES + P        # 4736
B2_COLS = B2_IDB + P        # 4864
# gF (f32): resm2 (sb-major) | b1cb
F_RES = 0
F_B1 = SB * D               # 4096
F_COLS = F_B1 + DB          # 4100

_COMPILED = None


def _build():
    import concourse.bacc as bacc
    import concourse.tile as tile
    import concourse.mybir as mybir

    f32 = mybir.dt.float32
    bf16 = mybir.dt.bfloat16
    AF = mybir.ActivationFunctionType
    ALU = mybir.AluOpType

    nc = bacc.Bacc("TRN2", target_bir_lowering=False, debug=False,
                   num_devices=N_CORES)

    packA = nc.dram_tensor("packA", [P, A_COLS], bf16, kind="ExternalInput").ap()
    packB1 = nc.dram_tensor("packB1", [P, B1_COLS], bf16, kind="ExternalInput").ap()
    packB2 = nc.dram_tensor("packB2", [P, B2_COLS], bf16, kind="ExternalInput").ap()
    packF = nc.dram_tensor("packF", [P, F_COLS], f32, kind="ExternalInput").ap()
    out = nc.dram_tensor("out", [S, D], f32, kind="ExternalOutput").ap()

    with tile.TileContext(nc) as tc:
        import contextlib
        ctx = contextlib.ExitStack()
        with ctx:
            big = ctx.enter_context(tc.tile_pool(name="big", bufs=1))
            psA = ctx.enter_context(tc.tile_pool(name="psA", bufs=4, space="PSUM"))
            psT = ctx.enter_context(tc.tile_pool(name="psT", bufs=4, space="PSUM"))
            work = ctx.enter_context(tc.tile_pool(name="work", bufs=3))

            # all loads on the Sync HWDGE queue: FIFO order = priority order,
            # so phase A's operands stream at full bandwidth first
            gA = big.tile([P, A_COLS], bf16, name="gA")
            CHUNKS = [(A_ATG0, A_INPT0),                      # aTg s-half 0
                      (A_INPT0, A_INPT0 + DB * P),            # inpT tb0
                      (A_INPT0 + DB * P, A_ATG1),             # inpT tb1-3
                      (A_INPT4, A_COLS),                      # inpT tb4-7
                      (A_ATG1, A_INPT4)]                      # aTg s-half 1
            for lo, hi in CHUNKS:
                nc.sync.dma_start(gA[:, lo:hi], packA[:, lo:hi])
            gB1 = big.tile([P, B1_COLS], bf16, name="gB1")
            nc.sync.dma_start(gB1[:], packB1[:])
            gF = big.tile([P, F_COLS], f32, name="gF")
            nc.sync.dma_start(gF[:], packF[:])
            gB2 = big.tile([P, B2_COLS], bf16, name="gB2")
            nc.sync.dma_start(gB2[:], packB2[:])

            def inpT(tb, db):       # [P, P]: lhsT for t-block tb, d-block db
                base = A_INPT0 if tb < 4 else A_INPT4
                off = base + (tb % 4) * DB * P + db * P
                return gA[:, off: off + P]

            def aTg(h, db):         # [P, NF]: aTg[d-block db, s-half h]
                base = A_ATG0 if h == 0 else A_ATG1
                off = base + db * NF
                return gA[:, off: off + NF]

            def inpN(sb):           # [P, D]
                return gB1[:, sb * D: (sb + 1) * D]

            def w1th(db):           # [P, D]
                return gB2[:, B2_W1 + db * D: B2_W1 + (db + 1) * D]

            def w2t(eb):
                return gB2[:, B2_W2 + eb * D: B2_W2 + (eb + 1) * D]

            b2row = gB2[0:1, B2_B2: B2_B2 + D]
            onesrow = gB2[0:1, B2_ONES: B2_ONES + P]
            identb = gB2[:, B2_IDB: B2_IDB + P]

            def resm2(sb):          # [P, D] f32
                return gF[:, F_RES + sb * D: F_RES + (sb + 1) * D]

            def b1col(eb):          # [P, 1] f32
                return gF[:, F_B1 + eb: F_B1 + eb + 1]

            # ---- phase A: attnT[t,s] = sum_d inpT[d,t] * aTg[d,s] --------
            attnT_sb = []
            for tb in range(SB):
                at_t = big.tile([P, S], bf16, name=f"attnT_sb{tb}")
                attnT_sb.append(at_t)
            gi = 0
            for h in range(SH):
                for tb in range(SB):
                    ps = psA.tile([P, NF], f32, name="psA_t", tag="psA")
                    for db in range(DB):
                        nc.tensor.matmul(
                            ps[:],
                            inpT(tb, db),
                            aTg(h, db),
                            start=(db == 0),
                            stop=(db == DB - 1),
                        )
                    dst = attnT_sb[tb][:, h * NF:(h + 1) * NF]
                    if gi % 2 == 0:
                        nc.scalar.activation(dst, ps[:], AF.Copy)
                    else:
                        nc.vector.tensor_copy(dst, ps[:])
                    gi += 1

            # ---- phase B + C interleaved ---------------------------------
            # B: F2 = 2*ffn_inp = attnT.T @ inpN + resm2  (add on DVE)
            # C: ffnT2 = F2.T via PE transposes into one [P,512] PSUM tile
            #    per s-block, evacuated with a single strided copy
            F2_sb = []
            F2b_sb = []
            for sb in range(SB):
                f2 = big.tile([P, D], f32, name=f"F2_sb{sb}")
                F2_sb.append(f2)
                f2b = big.tile([P, D], bf16, name=f"F2b_sb{sb}")
                F2b_sb.append(f2b)
            ffnT2 = big.tile([P, DB * S], bf16, name="ffnT2")   # db-major [db*S + s]
            ffnT2_v = ffnT2.rearrange("p (db s) -> p db s", db=DB)
            for sb in range(SB):
                ps = psA.tile([P, NF], f32, name="psB_t", tag="psA")
                for tb in range(SB):
                    nc.tensor.matmul(
                        ps[:],
                        attnT_sb[tb][:, sb * P:(sb + 1) * P],
                        inpN(tb),
                        start=(tb == 0),
                        stop=(tb == SB - 1),
                    )
                nc.vector.tensor_add(F2_sb[sb][:], ps[:], resm2(sb))
                nc.vector.tensor_copy(F2b_sb[sb][:], F2_sb[sb][:])
                for db in range(DB):
                    pst = psT.tile([P, P], bf16, name="psT_t", tag="psT")
                    nc.tensor.transpose(
                        pst[:], F2b_sb[sb][:, db * P:(db + 1) * P], identb)
                    dst = ffnT2[:, db * S + sb * P: db * S + (sb + 1) * P]
                    if (sb + db) % 2 == 0:
                        nc.scalar.activation(dst, pst[:], AF.Copy)
                    else:
                        nc.vector.tensor_copy(dst, pst[:])

            # ---- phase D: o1T = relu(w1th.T @ ffnT2 + b1) [e, s] ---------
            o1T_sb = []
            for eb in range(DB):
                t = big.tile([P, S], bf16, name=f"o1T_sb{eb}")
                o1T_sb.append(t)
            for eb in range(DB):
                for h in range(SH):
                    ps = psA.tile([P, NF], f32, name="psD_t", tag="psA")
                    for db in range(DB):
                        nc.tensor.matmul(
                            ps[:],
                            w1th(db)[:, eb * P:(eb + 1) * P],
                            ffnT2[:, db * S + h * NF: db * S + (h + 1) * NF],
                            start=(db == 0),
                            stop=(db == DB - 1),
                        )
                    nc.scalar.activation(
                        o1T_sb[eb][:, h * NF:(h + 1) * NF], ps[:], AF.Relu,
                        bias=b1col(eb), scale=1.0)

            # ---- phase E: o2, final, normalize, store --------------------
            for sb in range(SB):
                ps = psA.tile([P, NF], f32, name="psE_t", tag="psA")
                nc.tensor.matmul(ps[:], onesrow, b2row, start=True, stop=False)
                for eb in range(DB):
                    nc.tensor.matmul(
                        ps[:],
                        o1T_sb[eb][:, sb * P:(sb + 1) * P],
                        w2t(eb),
                        start=False,
                        stop=(eb == DB - 1),
                    )
                o2 = work.tile([P, D], f32, name="o2_t", tag="o2")
                nc.scalar.activation(o2[:], ps[:], AF.Relu)
                fin = work.tile([P, D], f32, name="fin_t", tag="fin")
                nc.vector.tensor_add(fin[:], o2[:], F2_sb[sb][:])
                sq = work.tile([P, D], f32, name="sq_t", tag="sq")
                ss = work.tile([P, 1], f32, name="ss_t", tag="ss")
                nc.scalar.activation(sq[:], fin[:], AF.Square,
                                     accum_out=ss[:])
                rn = work.tile([P, 1], f32, name="rn_t", tag="rn")
                nc.scalar.activation(rn[:], ss[:], AF.Sqrt)
                rr = work.tile([P, 1], f32, name="rr_t", tag="rr")
                nc.vector.reciprocal(rr[:], rn[:])
                ot = work.tile([P, D], f32, name="ot_t", tag="ot")
                nc.vector.tensor_scalar_mul(ot[:], fin[:], rr[:])
                if sb % 2 == 0:
                    nc.sync.dma_start(out[sb * P:(sb + 1) * P, :], ot[:])
                else:
                    nc.scalar.dma_start(out[sb * P:(sb + 1) * P, :], ot[:])

    nc.compile()
    return nc


def _get_compiled():
    global _COMPILED
    if _COMPILED is None:
        _COMPILED = _build()
    return _COMPILED


def _host_prep(inp, inp_len, aspect, w1, b1, w2, b2):
    inp = np.asarray(inp, dtype=np.float32)
    aspect = np.asarray(aspect, dtype=np.float32)
    inp_len = np.asarray(inp_len, dtype=np.float32)
    w1 = np.asarray(w1, dtype=np.float32)
    b1 = np.asarray(b1, dtype=np.float32)
    w2 = np.asarray(w2, dtype=np.float32)
    b2 = np.asarray(b2, dtype=np.float32)

    packB2 = np.zeros((P, B2_COLS), dtype=BF16)
    w1th = (w1.T * 0.5).astype(BF16)                 # [d, e]
    w2t = w2.T.astype(BF16)                          # [e, f]
    for db in range(DB):
        packB2[:, B2_W1 + db * D: B2_W1 + (db + 1) * D] = \
            w1th[db * P:(db + 1) * P, :]
        packB2[:, B2_W2 + db * D: B2_W2 + (db + 1) * D] = \
            w2t[db * P:(db + 1) * P, :]
    packB2[0, B2_B2: B2_B2 + D] = b2.astype(BF16)
    packB2[0, B2_ONES: B2_ONES + P] = np.ones(P, dtype=BF16)
    packB2[:, B2_IDB: B2_IDB + P] = np.eye(P).astype(BF16)

    b1cb = b1.reshape(DB, P).T.astype(np.float32)    # [P, DB]

    in_maps = []
    for bidx in range(B):
        x = inp[bidx].astype(np.float64)             # [S, D]
        a = aspect[bidx].astype(np.float64)
        ln = float(inp_len[bidx])
        scale = np.sqrt(ln)
        mask = (np.arange(S) < int(ln)).astype(np.float64)
        rowsum = a @ x.sum(axis=0)
        g = mask / (mask * rowsum + 1e-4 * scale)
        aTg2 = ((a * (2.0 * g)[:, None]).T).astype(BF16)   # [D, S]
        xT = x.T.astype(BF16)                              # [D, S]
        resm2 = 2.0 * (x + a) * mask[:, None]              # [S, D]

        pA = np.empty((P, A_COLS), dtype=BF16)
        for h, base in ((0, A_ATG0), (1, A_ATG1)):
            for db in range(DB):
                pA[:, base + db * NF: base + (db + 1) * NF] = \
                    aTg2[db * P:(db + 1) * P, h * NF:(h + 1) * NF]
        for tb in range(SB):
            base = (A_INPT0 if tb < 4 else A_INPT4) + (tb % 4) * DB * P
            for db in range(DB):
                pA[:, base + db * P: base + (db + 1) * P] = \
                    xT[db * P:(db + 1) * P, tb * P:(tb + 1) * P]

        pB1 = np.empty((P, B1_COLS), dtype=BF16)
        xb = x.astype(BF16)
        for sb in range(SB):
            pB1[:, sb * D:(sb + 1) * D] = xb[sb * P:(sb + 1) * P, :]

        pF = np.zeros((P, F_COLS), dtype=np.float32)
        r32 = resm2.astype(np.float32)
        for sb in range(SB):
            pF[:, F_RES + sb * D: F_RES + (sb + 1) * D] = \
                r32[sb * P:(sb + 1) * P, :]
        pF[:, F_B1: F_B1 + DB] = b1cb

        in_maps.append({"packA": pA, "packB1": pB1, "packB2": packB2,
                        "packF": pF})
    return in_maps


def kernel(inp, inp_len, aspect, w1, b1, w2, b2):
    from concourse.bass_utils import run_bass_kernel_spmd

    nc = _get_compiled()
    in_maps = _host_prep(inp, inp_len, aspect, w1, b1, w2, b2)
    res = run_bass_kernel_spmd(nc, in_maps, core_ids=list(range(N_CORES)))
    return np.stack([res.results[i]["out"] for i in range(N_CORES)], axis=0)
